# revision 9
# baseline (speedup 1.0000x reference)
"""CrossNet kernel for Trainium2 (8 NeuronCores, pure data parallel over batch).

Math: reference computes, for i in 0..2:
    s_i = x_k @ w_i          (per-row dot)
    x_k = x * s_i + b_i + x_k
and returns the three intermediate x_k.

Flattened (by induction):  x_k = x * S_k + B_k + x, with
    t_i = 1 + S_{i+1}:  c_j = x @ w_j,  t_0 = 1 + c_0,
    s_j = t_{j-1} * c_j + d_j  (d_j = cumb_{j-1} @ w_j, host),
    t_j = t_{j-1} + s_j,  out_i = x * t_i + cumb_i.

v7: measured-rate engine plan (f16 [128,4096] pass): DVE TT 2.28us,
STT(+accum) 4.42us, TS 1.28us; Scalar ACT 3.7us (full) / 2.0us (half,
also PSUM->SBUF); PE ~0.59us per 512-col MM. Native TENSOR_TENSOR_REDUCE
crashes the HW; custom DVE ops run 1x — both dead ends. DMA ~425 GB/s
per core aggregate across the sync/scalar/gpsimd dynamic queues.

Changes vs v6:
 - wb/cbb const tiles load via broadcast-AP DMA from [1,N] rows (drops
   the host-replicated 6.2 MB wrep/cbrep inputs; read side is 8KB).
 - dots: 'v' = STT+accum (DVE only), 's' = TT product + Scalar ACT-accum.
 - outs: 'd' = TS + TT-add (DVE), 'e' = Scalar ACT-mul + DVE TT-add,
   'p' = PE diag(t)+ones*cumb matmuls + Scalar PSUM copies.
 - out DMA + loads issue on sync/gpsimd queues, keeping Scalar for ACT.
 - outs emitted per (tile, i) right after t_i: steady out-DMA flow.
"""

import os

import numpy as np

B, N, ORDER, NCORES = 4096, 4096, 3, 8
ROWS = B // NCORES  # 512 rows per core
P = 128
NT = ROWS // P  # 4 partition-tiles per core
HALF = 2048

# per-(tile,dot) impl: v=STT+accum(DVE), s=TT+ACT(Scalar reduce)
DOTS = os.environ.get("CK_DOTS", "vss" "sss" "vss" "vss")
# per-(tile,out) impl: d=TS+TT(DVE), e=ACT-mul+TT, p=PE, g=ACT-mul+GpSimd-add
LANES = os.environ.get("CK_LANES", "ppp" "pdp" "dpd" "pdh")
# DMA issue queues: s=sync, a=scalar, g=gpsimd
X_Q = os.environ.get("CK_X_Q", "sgsg")          # per x tile
OUT_Q = os.environ.get("CK_OUT_Q", "sgsgsgsgsgsg")  # per out (tile*3+i)
CONST_Q = os.environ.get("CK_CONST_Q", "g")     # wb1/wb2/cbb loads
XBUFS = int(os.environ.get("CK_XBUFS", "4"))
OBUFS = int(os.environ.get("CK_OBUFS", "6"))
TBUFS = int(os.environ.get("CK_TBUFS", "3"))
PBUFS = int(os.environ.get("CK_PBUFS", "2"))

_prog_cache = {}


def _expand(s, n=None):
    s = "".join(c for c in s if not c.isspace())
    n = n or ORDER * NT
    if len(s) == 1:
        s = s * n
    if len(s) == ORDER and n == ORDER * NT:
        s = s * NT
    assert len(s) == n, s
    return s


def _build_program():
    from contextlib import ExitStack

    import concourse.bacc as bacc
    import concourse.mybir as mybir
    import concourse.tile as tile

    f32 = mybir.dt.float32
    f16 = mybir.dt.float16
    Alu = mybir.AluOpType
    Act = mybir.ActivationFunctionType

    dots = _expand(DOTS)
    lanes = _expand(LANES)
    x_q = _expand(X_Q, NT)
    out_q = _expand(OUT_Q)

    nc = bacc.Bacc("TRN2")
    xs = nc.dram_tensor("xs", [ROWS, N], f16, kind="ExternalInput")
    w3 = nc.dram_tensor("w3", [ORDER, N], f16, kind="ExternalInput")
    cb3 = nc.dram_tensor("cb3", [ORDER, N], f16, kind="ExternalInput")
    dd = nc.dram_tensor("dd", [P, ORDER], f32, kind="ExternalInput")
    eye = nc.dram_tensor("eye", [P, P], f16, kind="ExternalInput")
    out = nc.dram_tensor("out", [ORDER, ROWS, N], f16, kind="ExternalOutput")

    need_cbb = sorted(
        {
            q
            for k in range(NT)
            for q, c in enumerate(lanes[3 * k : 3 * k + 3])
            if c in "gdeh"
        }
    )
    any_pe_out = any(c == "p" for c in lanes)

    def q_eng(c):
        return {"s": nc.sync, "a": nc.scalar, "g": nc.gpsimd}[c]

    cq = q_eng(_expand(CONST_Q, 1))

    with ExitStack() as ctx:
        tc = ctx.enter_context(tile.TileContext(nc))
        consts = ctx.enter_context(tc.tile_pool(name="consts", bufs=1))
        xpool = ctx.enter_context(tc.tile_pool(name="xpool", bufs=XBUFS))
        small = ctx.enter_context(tc.tile_pool(name="small", bufs=4))
        opool = ctx.enter_context(tc.tile_pool(name="opool", bufs=OBUFS))
        tpool = ctx.enter_context(tc.tile_pool(name="tpool", bufs=TBUFS))
        psum = ctx.enter_context(tc.tile_pool(name="psum", bufs=PBUFS, space="PSUM"))
        scratchpool = ctx.enter_context(tc.tile_pool(name="scratch", bufs=1))

        wb = [
            consts.tile([P, N], f16, tag=f"wb{j}", name=f"wb{j}")
            for j in range(ORDER)
        ]
        cbb = {
            i: consts.tile([P, N], f16, tag=f"cbb{i}", name=f"cbb{i}")
            for i in need_cbb
        }
        dd_t = consts.tile([P, ORDER], f32, tag="dd")
        eye_t = consts.tile([P, P], f16, tag="eye")

        # Queue FIFO order == emission order. Lead-in priority: wb0 and x0
        # halves split across the sync/gpsimd queues so dot0 of tile 0 can
        # start as early as the ~11us queue warm-up allows; x tiles BEFORE
        # bulk consts (a tile stuck behind consts stalls DVE for ~20us).
        xts = [
            xpool.tile([P, N], f16, tag="x", name=f"x{k}") for k in range(NT)
        ]
        QU = N // 4
        nc.sync.dma_start(
            out=wb[0][:, :HALF], in_=w3[0:1, :HALF].to_broadcast([P, HALF])
        )
        nc.gpsimd.dma_start(
            out=wb[0][:, HALF:], in_=w3[0:1, HALF:].to_broadcast([P, HALF])
        )
        nc.scalar.dma_start(out=dd_t, in_=dd[:, :])
        nc.sync.dma_start(out=xts[0][:, :QU], in_=xs[:P, :QU])
        nc.scalar.dma_start(out=xts[0][:, QU : 2 * QU], in_=xs[:P, QU : 2 * QU])
        nc.gpsimd.dma_start(
            out=xts[0][:, 2 * QU : 3 * QU], in_=xs[:P, 2 * QU : 3 * QU]
        )
        nc.scalar.dma_start(out=xts[0][:, 3 * QU :], in_=xs[:P, 3 * QU :])
        cpack = None
        if any_pe_out:
            cpack = consts.tile([2 * 32 + 1, N], f16, tag="cpack")
            nc.scalar.dma_start(out=eye_t, in_=eye[:, :])
            for j in range(ORDER):
                nc.scalar.dma_start(
                    out=cpack[32 * j : 32 * j + 1, :], in_=cb3[j : j + 1, :]
                )
        nc.sync.dma_start(out=wb[1], in_=w3[1:2, :].to_broadcast([P, N]))
        nc.sync.dma_start(out=xts[1], in_=xs[P : 2 * P, :])
        nc.gpsimd.dma_start(out=wb[2], in_=w3[2:3, :].to_broadcast([P, N]))
        nc.gpsimd.dma_start(out=xts[2], in_=xs[2 * P : 3 * P, :])
        for n_, i in enumerate(need_cbb):
            q_eng("sg"[n_ % 2]).dma_start(
                out=cbb[i], in_=cb3[i : i + 1, :].to_broadcast([P, N])
            )
        nc.gpsimd.dma_start(out=xts[3], in_=xs[3 * P : 4 * P, :])
        opack = consts.tile([2 * 32 + 1, P], f16, tag="opack")
        nc.vector.memset(opack, 1.0)

        def row_of(pack, j):
            return pack[32 * j : 32 * j + 1, :]

        def emit_dot(k, j, x_t, cs, ts):
            scratch = scratchpool.tile([P, N], f16, tag="scr")
            cj = small.tile([P, 1], f32, tag=f"c{j}")
            if k == 0 and j == 0 and dots[0] == "v":
                cp = [
                    small.tile([P, 1], f32, tag=f"cp{h}", name=f"cp{h}")
                    for h in range(2)
                ]
                for h in range(2):
                    sl = slice(h * HALF, (h + 1) * HALF)
                    nc.vector.scalar_tensor_tensor(
                        out=scratch[:, sl],
                        in0=x_t[:, sl],
                        scalar=1.0,
                        in1=wb[0][:, sl],
                        op0=Alu.mult,
                        op1=Alu.mult,
                        accum_out=cp[h],
                    )
                nc.vector.tensor_add(cj, cp[0], cp[1])
            elif dots[3 * k + j] == "s":
                prod = tpool.tile([P, N], f16, tag="prod")
                nc.vector.tensor_tensor(prod, x_t, wb[j], Alu.mult)
                nc.scalar.activation(prod, prod, Act.Copy, accum_out=cj)
            else:
                nc.vector.scalar_tensor_tensor(
                    out=scratch,
                    in0=x_t,
                    scalar=1.0,
                    in1=wb[j],
                    op0=Alu.mult,
                    op1=Alu.mult,
                    accum_out=cj,
                )
            cs.append(cj)
            if j == 0:
                t1 = small.tile([P, 1], f32, tag="t0")
                nc.vector.tensor_scalar_add(t1, cs[0], 1.0)
                ts.append(t1)
            else:
                si = small.tile([P, 1], f32, tag=f"s{j}")
                nc.vector.tensor_scalar(
                    out=si,
                    in0=cs[j],
                    scalar1=ts[j - 1],
                    scalar2=dd_t[:, j : j + 1],
                    op0=Alu.mult,
                    op1=Alu.add,
                )
                ti = small.tile([P, 1], f32, tag=f"t{j}")
                nc.vector.tensor_add(ti, ts[j - 1], si)
                ts.append(ti)

        def emit_out(k, i, x_t, ti):
            rows = slice(k * P, (k + 1) * P)
            lane = lanes[3 * k + i]
            ob = opool.tile([P, N], f16, tag="ob")
            if lane == "h":
                for hh in range(2):
                    sl = slice(hh * HALF, (hh + 1) * HALF)
                    nc.vector.tensor_scalar(
                        out=ob[:, sl], in0=x_t[:, sl], scalar1=ti,
                        scalar2=None, op0=Alu.mult
                    )
                    nc.vector.tensor_add(ob[:, sl], ob[:, sl], cbb[i][:, sl])
                    q_eng("sg"[hh]).dma_start(
                        out=out[i, rows, sl], in_=ob[:, sl]
                    )
                return
            if lane == "d":
                nc.vector.tensor_scalar(
                    out=ob, in0=x_t, scalar1=ti, scalar2=None, op0=Alu.mult
                )
                nc.vector.tensor_add(ob, ob, cbb[i])
            elif lane == "e":
                nc.scalar.mul(ob, x_t, ti)
                nc.vector.tensor_add(ob, ob, cbb[i])
            elif lane == "g":
                tmp = tpool.tile([P, N], f16, tag="prod")
                nc.scalar.mul(tmp, x_t, ti)
                nc.gpsimd.tensor_add(ob, tmp, cbb[i])
            else:  # 'p'
                dg = small.tile([P, P], f16, tag=f"dg{i}")
                nc.vector.tensor_scalar_mul(dg, eye_t, ti)
                for h in range(N // HALF):
                    pt = psum.tile([P, HALF], f32, tag="ps")
                    for q in range(HALF // 512):
                        sl = slice(h * HALF + q * 512, h * HALF + (q + 1) * 512)
                        nc.tensor.matmul(
                            pt[:, q * 512 : (q + 1) * 512],
                            lhsT=row_of(opack, i),
                            rhs=row_of(cpack, i)[:, sl],
                            start=True,
                            stop=False,
                        )
                    for q in range(HALF // 512):
                        sl = slice(h * HALF + q * 512, h * HALF + (q + 1) * 512)
                        nc.tensor.matmul(
                            pt[:, q * 512 : (q + 1) * 512],
                            lhsT=dg,
                            rhs=x_t[:, sl],
                            start=False,
                            stop=True,
                        )
                    nc.scalar.copy(ob[:, h * HALF : (h + 1) * HALF], pt)
            q_eng(out_q[3 * k + i]).dma_start(out=out[i, rows, :], in_=ob)

        # Software pipelining: every out trails its dot by ~a full tile of
        # queue work, so out ops never wait on the ACT-reduce recurrence
        # round-trip (v6 lesson: zero mid-span DVE stalls).
        ts_all = {}
        for k in range(NT):
            cs, ts = [], []
            for j in range(ORDER):
                emit_dot(k, j, xts[k], cs, ts)
            ts_all[k] = ts
            if k == 0:
                emit_out(0, 0, xts[0], ts_all[0][0])
            else:
                emit_out(k - 1, 1, xts[k - 1], ts_all[k - 1][1])
                emit_out(k - 1, 2, xts[k - 1], ts_all[k - 1][2])
                emit_out(k, 0, xts[k], ts_all[k][0])
        emit_out(NT - 1, 1, xts[NT - 1], ts_all[NT - 1][1])
        emit_out(NT - 1, 2, xts[NT - 1], ts_all[NT - 1][2])

    nc.finalize()
    return nc


def _get_program():
    if "nc" not in _prog_cache:
        _prog_cache["nc"] = _build_program()
    return _prog_cache["nc"]


def _prep_inputs(x, w, b):
    x16 = np.asarray(x, dtype=np.float32).astype(np.float16)
    w_r = np.asarray(w, dtype=np.float32).reshape(ORDER, N).astype(np.float16)
    b_r = np.asarray(b, dtype=np.float32).reshape(ORDER, N)
    cumb = np.cumsum(b_r, axis=0).astype(np.float16)  # cumb[i] = b_0 + ... + b_i
    d = np.zeros(ORDER, dtype=np.float64)
    for i in range(1, ORDER):
        d[i] = cumb[i - 1].astype(np.float64) @ w_r[i].astype(np.float64)
    dd = np.tile(d.astype(np.float32)[None, :], (P, 1))
    eye = np.eye(P, dtype=np.float16)

    shared = {
        "w3": np.ascontiguousarray(w_r),
        "cb3": np.ascontiguousarray(cumb),
        "dd": np.ascontiguousarray(dd),
        "eye": eye,
    }
    in_maps = []
    for c in range(NCORES):
        m = dict(shared)
        m["xs"] = np.ascontiguousarray(x16[c * ROWS : (c + 1) * ROWS, :])
        in_maps.append(m)
    return in_maps


def _run(x, w, b, trace=False):
    from concourse.bass_utils import run_bass_kernel_spmd

    nc = _get_program()
    in_maps = _prep_inputs(x, w, b)
    res = run_bass_kernel_spmd(nc, in_maps, core_ids=list(range(NCORES)), trace=trace)
    outs = [np.asarray(r["out"]) for r in res.results]  # each [ORDER, ROWS, N] f16
    full = np.concatenate(outs, axis=1)  # [ORDER, B, N]
    return (
        tuple(np.ascontiguousarray(full[i]).astype(np.float32) for i in range(ORDER)),
        res,
    )


def kernel(x, w, b):
    outs, _ = _run(x, w, b, trace=False)
    return outs


# revision 10
# speedup vs baseline: 1.0571x; 1.0571x over previous
"""CrossNet kernel for Trainium2 (8 NeuronCores, pure data parallel over batch).

Math: reference computes, for i in 0..2:
    s_i = x_k @ w_i          (per-row dot)
    x_k = x * s_i + b_i + x_k
and returns the three intermediate x_k.

Flattened (by induction):  x_k = x * S_k + B_k + x, with
    t_i = 1 + S_{i+1}:  c_j = x @ w_j,  t_0 = 1 + c_0,
    s_j = t_{j-1} * c_j + d_j  (d_j = cumb_{j-1} @ w_j, host),
    t_j = t_{j-1} + s_j,  out_i = x * t_i + cumb_i.

v7: measured-rate engine plan (f16 [128,4096] pass): DVE TT 2.28us,
STT(+accum) 4.42us, TS 1.28us; Scalar ACT 3.7us (full) / 2.0us (half,
also PSUM->SBUF); PE ~0.59us per 512-col MM. Native TENSOR_TENSOR_REDUCE
crashes the HW; custom DVE ops run 1x — both dead ends. DMA ~425 GB/s
per core aggregate across the sync/scalar/gpsimd dynamic queues.

Changes vs v6:
 - wb/cbb const tiles load via broadcast-AP DMA from [1,N] rows (drops
   the host-replicated 6.2 MB wrep/cbrep inputs; read side is 8KB).
 - dots: 'v' = STT+accum (DVE only), 's' = TT product + Scalar ACT-accum.
 - outs: 'd' = TS + TT-add (DVE), 'e' = Scalar ACT-mul + DVE TT-add,
   'p' = PE diag(t)+ones*cumb matmuls + Scalar PSUM copies.
 - out DMA + loads issue on sync/gpsimd queues, keeping Scalar for ACT.
 - outs emitted per (tile, i) right after t_i: steady out-DMA flow.
"""

import os

import numpy as np

B, N, ORDER, NCORES = 4096, 4096, 3, 8
ROWS = B // NCORES  # 512 rows per core
P = 128
NT = ROWS // P  # 4 partition-tiles per core
HALF = 2048

# per-(tile,dot) impl: v=STT+accum(DVE), s=TT+ACT(Scalar reduce)
DOTS = os.environ.get("CK_DOTS", "vss" "sss" "vss" "vss")
# per-(tile,out) impl: d=TS+TT(DVE), e=ACT-mul+TT, p=PE, g=ACT-mul+GpSimd-add
LANES = os.environ.get("CK_LANES", "ppp" "pdp" "dpd" "pdh")
# DMA issue queues: s=sync, a=scalar, g=gpsimd
X_Q = os.environ.get("CK_X_Q", "sgsg")          # per x tile
OUT_Q = os.environ.get("CK_OUT_Q", "sgsgsgsgsgsg")  # per out (tile*3+i)
CONST_Q = os.environ.get("CK_CONST_Q", "g")     # wb1/wb2/cbb loads
XBUFS = int(os.environ.get("CK_XBUFS", "4"))
OBUFS = int(os.environ.get("CK_OBUFS", "6"))
TBUFS = int(os.environ.get("CK_TBUFS", "3"))
PBUFS = int(os.environ.get("CK_PBUFS", "2"))

_prog_cache = {}


def _expand(s, n=None):
    s = "".join(c for c in s if not c.isspace())
    n = n or ORDER * NT
    if len(s) == 1:
        s = s * n
    if len(s) == ORDER and n == ORDER * NT:
        s = s * NT
    assert len(s) == n, s
    return s


def _build_program():
    from contextlib import ExitStack

    import concourse.bacc as bacc
    import concourse.mybir as mybir
    import concourse.tile as tile

    f32 = mybir.dt.float32
    f16 = mybir.dt.float16
    Alu = mybir.AluOpType
    Act = mybir.ActivationFunctionType

    dots = _expand(DOTS)
    lanes = _expand(LANES)
    x_q = _expand(X_Q, NT)
    out_q = _expand(OUT_Q)

    nc = bacc.Bacc("TRN2")
    xs = nc.dram_tensor("xs", [ROWS, N], f16, kind="ExternalInput")
    w3 = nc.dram_tensor("w3", [ORDER, N], f16, kind="ExternalInput")
    cb3 = nc.dram_tensor("cb3", [ORDER, N], f16, kind="ExternalInput")
    dd = nc.dram_tensor("dd", [P, ORDER], f32, kind="ExternalInput")
    eye = nc.dram_tensor("eye", [P, P], f16, kind="ExternalInput")
    out = nc.dram_tensor("out", [ORDER, ROWS, N], f16, kind="ExternalOutput")

    need_cbb = sorted(
        {
            q
            for k in range(NT)
            for q, c in enumerate(lanes[3 * k : 3 * k + 3])
            if c in "gdeh"
        }
    )
    any_pe_out = any(c == "p" for c in lanes)

    def q_eng(c):
        return {"s": nc.sync, "a": nc.scalar, "g": nc.gpsimd}[c]

    cq = q_eng(_expand(CONST_Q, 1))

    with ExitStack() as ctx:
        tc = ctx.enter_context(tile.TileContext(nc))
        consts = ctx.enter_context(tc.tile_pool(name="consts", bufs=1))
        xpool = ctx.enter_context(tc.tile_pool(name="xpool", bufs=XBUFS))
        small = ctx.enter_context(tc.tile_pool(name="small", bufs=4))
        opool = ctx.enter_context(tc.tile_pool(name="opool", bufs=OBUFS))
        tpool = ctx.enter_context(tc.tile_pool(name="tpool", bufs=TBUFS))
        psum = ctx.enter_context(tc.tile_pool(name="psum", bufs=PBUFS, space="PSUM"))
        scratchpool = ctx.enter_context(tc.tile_pool(name="scratch", bufs=1))

        wb = [
            consts.tile([P, N], f16, tag=f"wb{j}", name=f"wb{j}")
            for j in range(ORDER)
        ]
        cbb = {
            i: consts.tile([P, N], f16, tag=f"cbb{i}", name=f"cbb{i}")
            for i in need_cbb
        }
        dd_t = consts.tile([P, ORDER], f32, tag="dd")
        eye_t = consts.tile([P, P], f16, tag="eye")

        # Queue FIFO order == emission order. Lead-in priority: wb0 and x0
        # halves split across the sync/gpsimd queues so dot0 of tile 0 can
        # start as early as the ~11us queue warm-up allows; x tiles BEFORE
        # bulk consts (a tile stuck behind consts stalls DVE for ~20us).
        xts = [
            xpool.tile([P, N], f16, tag="x", name=f"x{k}") for k in range(NT)
        ]
        nc.sync.dma_start(
            out=wb[0][:, :HALF], in_=w3[0:1, :HALF].to_broadcast([P, HALF])
        )
        nc.gpsimd.dma_start(
            out=wb[0][:, HALF:], in_=w3[0:1, HALF:].to_broadcast([P, HALF])
        )
        nc.sync.dma_start(out=xts[0][:, :HALF], in_=xs[:P, :HALF])
        nc.gpsimd.dma_start(out=xts[0][:, HALF:], in_=xs[:P, HALF:])
        nc.scalar.dma_start(out=dd_t, in_=dd[:, :])
        cpack = None
        if any_pe_out:
            cpack = consts.tile([2 * 32 + 1, N], f16, tag="cpack")
            nc.scalar.dma_start(out=eye_t, in_=eye[:, :])
            for j in range(ORDER):
                nc.scalar.dma_start(
                    out=cpack[32 * j : 32 * j + 1, :], in_=cb3[j : j + 1, :]
                )
        nc.sync.dma_start(out=wb[1], in_=w3[1:2, :].to_broadcast([P, N]))
        nc.sync.dma_start(out=xts[1], in_=xs[P : 2 * P, :])
        nc.gpsimd.dma_start(out=wb[2], in_=w3[2:3, :].to_broadcast([P, N]))
        nc.gpsimd.dma_start(out=xts[2], in_=xs[2 * P : 3 * P, :])
        for n_, i in enumerate(need_cbb):
            q_eng("sg"[n_ % 2]).dma_start(
                out=cbb[i], in_=cb3[i : i + 1, :].to_broadcast([P, N])
            )
        nc.gpsimd.dma_start(out=xts[3], in_=xs[3 * P : 4 * P, :])
        opack = consts.tile([2 * 32 + 1, P], f16, tag="opack")
        nc.vector.memset(opack, 1.0)

        def row_of(pack, j):
            return pack[32 * j : 32 * j + 1, :]

        def emit_dot(k, j, x_t, cs, ts):
            scratch = scratchpool.tile([P, N], f16, tag="scr")
            cj = small.tile([P, 1], f32, tag=f"c{j}")
            if k == 0 and j == 0 and dots[0] == "v":
                cp = [
                    small.tile([P, 1], f32, tag=f"cp{h}", name=f"cp{h}")
                    for h in range(2)
                ]
                for h in range(2):
                    sl = slice(h * HALF, (h + 1) * HALF)
                    nc.vector.scalar_tensor_tensor(
                        out=scratch[:, sl],
                        in0=x_t[:, sl],
                        scalar=1.0,
                        in1=wb[0][:, sl],
                        op0=Alu.mult,
                        op1=Alu.mult,
                        accum_out=cp[h],
                    )
                nc.vector.tensor_add(cj, cp[0], cp[1])
            elif dots[3 * k + j] == "s":
                prod = tpool.tile([P, N], f16, tag="prod")
                nc.vector.tensor_tensor(prod, x_t, wb[j], Alu.mult)
                nc.scalar.activation(prod, prod, Act.Copy, accum_out=cj)
            else:
                nc.vector.scalar_tensor_tensor(
                    out=scratch,
                    in0=x_t,
                    scalar=1.0,
                    in1=wb[j],
                    op0=Alu.mult,
                    op1=Alu.mult,
                    accum_out=cj,
                )
            cs.append(cj)
            if j == 0:
                t1 = small.tile([P, 1], f32, tag="t0")
                nc.vector.tensor_scalar_add(t1, cs[0], 1.0)
                ts.append(t1)
            else:
                si = small.tile([P, 1], f32, tag=f"s{j}")
                nc.vector.tensor_scalar(
                    out=si,
                    in0=cs[j],
                    scalar1=ts[j - 1],
                    scalar2=dd_t[:, j : j + 1],
                    op0=Alu.mult,
                    op1=Alu.add,
                )
                ti = small.tile([P, 1], f32, tag=f"t{j}")
                nc.vector.tensor_add(ti, ts[j - 1], si)
                ts.append(ti)

        def emit_out(k, i, x_t, ti):
            rows = slice(k * P, (k + 1) * P)
            lane = lanes[3 * k + i]
            ob = opool.tile([P, N], f16, tag="ob")
            if lane == "h":
                for hh in range(2):
                    sl = slice(hh * HALF, (hh + 1) * HALF)
                    nc.vector.tensor_scalar(
                        out=ob[:, sl], in0=x_t[:, sl], scalar1=ti,
                        scalar2=None, op0=Alu.mult
                    )
                    nc.vector.tensor_add(ob[:, sl], ob[:, sl], cbb[i][:, sl])
                    q_eng("sg"[hh]).dma_start(
                        out=out[i, rows, sl], in_=ob[:, sl]
                    )
                return
            if lane == "d":
                nc.vector.tensor_scalar(
                    out=ob, in0=x_t, scalar1=ti, scalar2=None, op0=Alu.mult
                )
                nc.vector.tensor_add(ob, ob, cbb[i])
            elif lane == "e":
                nc.scalar.mul(ob, x_t, ti)
                nc.vector.tensor_add(ob, ob, cbb[i])
            elif lane == "g":
                tmp = tpool.tile([P, N], f16, tag="prod")
                nc.scalar.mul(tmp, x_t, ti)
                nc.gpsimd.tensor_add(ob, tmp, cbb[i])
            else:  # 'p'
                dg = small.tile([P, P], f16, tag=f"dg{i}")
                nc.vector.tensor_scalar_mul(dg, eye_t, ti)
                for h in range(N // HALF):
                    pt = psum.tile([P, HALF], f32, tag="ps")
                    for q in range(HALF // 512):
                        sl = slice(h * HALF + q * 512, h * HALF + (q + 1) * 512)
                        nc.tensor.matmul(
                            pt[:, q * 512 : (q + 1) * 512],
                            lhsT=row_of(opack, i),
                            rhs=row_of(cpack, i)[:, sl],
                            start=True,
                            stop=False,
                        )
                    for q in range(HALF // 512):
                        sl = slice(h * HALF + q * 512, h * HALF + (q + 1) * 512)
                        nc.tensor.matmul(
                            pt[:, q * 512 : (q + 1) * 512],
                            lhsT=dg,
                            rhs=x_t[:, sl],
                            start=False,
                            stop=True,
                        )
                    nc.scalar.copy(ob[:, h * HALF : (h + 1) * HALF], pt)
            q_eng(out_q[3 * k + i]).dma_start(out=out[i, rows, :], in_=ob)

        # Software pipelining: every out trails its dot by ~a full tile of
        # queue work, so out ops never wait on the ACT-reduce recurrence
        # round-trip (v6 lesson: zero mid-span DVE stalls).
        ts_all = {}
        for k in range(NT):
            cs, ts = [], []
            for j in range(ORDER):
                emit_dot(k, j, xts[k], cs, ts)
            ts_all[k] = ts
            if k == 0:
                emit_out(0, 0, xts[0], ts_all[0][0])
            else:
                emit_out(k - 1, 1, xts[k - 1], ts_all[k - 1][1])
                emit_out(k - 1, 2, xts[k - 1], ts_all[k - 1][2])
                emit_out(k, 0, xts[k], ts_all[k][0])
        emit_out(NT - 1, 1, xts[NT - 1], ts_all[NT - 1][1])
        emit_out(NT - 1, 2, xts[NT - 1], ts_all[NT - 1][2])

    nc.finalize()
    return nc


def _get_program():
    if "nc" not in _prog_cache:
        _prog_cache["nc"] = _build_program()
    return _prog_cache["nc"]


def _prep_inputs(x, w, b):
    x16 = np.asarray(x, dtype=np.float32).astype(np.float16)
    w_r = np.asarray(w, dtype=np.float32).reshape(ORDER, N).astype(np.float16)
    b_r = np.asarray(b, dtype=np.float32).reshape(ORDER, N)
    cumb = np.cumsum(b_r, axis=0).astype(np.float16)  # cumb[i] = b_0 + ... + b_i
    d = np.zeros(ORDER, dtype=np.float64)
    for i in range(1, ORDER):
        d[i] = cumb[i - 1].astype(np.float64) @ w_r[i].astype(np.float64)
    dd = np.tile(d.astype(np.float32)[None, :], (P, 1))
    eye = np.eye(P, dtype=np.float16)

    shared = {
        "w3": np.ascontiguousarray(w_r),
        "cb3": np.ascontiguousarray(cumb),
        "dd": np.ascontiguousarray(dd),
        "eye": eye,
    }
    in_maps = []
    for c in range(NCORES):
        m = dict(shared)
        m["xs"] = np.ascontiguousarray(x16[c * ROWS : (c + 1) * ROWS, :])
        in_maps.append(m)
    return in_maps


def _run(x, w, b, trace=False):
    from concourse.bass_utils import run_bass_kernel_spmd

    nc = _get_program()
    in_maps = _prep_inputs(x, w, b)
    res = run_bass_kernel_spmd(nc, in_maps, core_ids=list(range(NCORES)), trace=trace)
    outs = [np.asarray(r["out"]) for r in res.results]  # each [ORDER, ROWS, N] f16
    full = np.concatenate(outs, axis=1)  # [ORDER, B, N]
    return (
        tuple(np.ascontiguousarray(full[i]).astype(np.float32) for i in range(ORDER)),
        res,
    )


def kernel(x, w, b):
    outs, _ = _run(x, w, b, trace=False)
    return outs


# revision 11
# speedup vs baseline: 1.1125x; 1.0524x over previous
"""CrossNet kernel for Trainium2 (8 NeuronCores, pure data parallel over batch).

Math: reference computes, for i in 0..2:
    s_i = x_k @ w_i          (per-row dot)
    x_k = x * s_i + b_i + x_k
and returns the three intermediate x_k.

Flattened (by induction):  x_k = x * S_k + B_k + x, with
    t_i = 1 + S_{i+1}:  c_j = x @ w_j,  t_0 = 1 + c_0,
    s_j = t_{j-1} * c_j + d_j  (d_j = cumb_{j-1} @ w_j, host),
    t_j = t_{j-1} + s_j,  out_i = x * t_i + cumb_i.

v7: measured-rate engine plan (f16 [128,4096] pass): DVE TT 2.28us,
STT(+accum) 4.42us, TS 1.28us; Scalar ACT 3.7us (full) / 2.0us (half,
also PSUM->SBUF); PE ~0.59us per 512-col MM. Native TENSOR_TENSOR_REDUCE
crashes the HW; custom DVE ops run 1x — both dead ends. DMA ~425 GB/s
per core aggregate across the sync/scalar/gpsimd dynamic queues.

Changes vs v6:
 - wb/cbb const tiles load via broadcast-AP DMA from [1,N] rows (drops
   the host-replicated 6.2 MB wrep/cbrep inputs; read side is 8KB).
 - dots: 'v' = STT+accum (DVE only), 's' = TT product + Scalar ACT-accum.
 - outs: 'd' = TS + TT-add (DVE), 'e' = Scalar ACT-mul + DVE TT-add,
   'p' = PE diag(t)+ones*cumb matmuls + Scalar PSUM copies.
 - out DMA + loads issue on sync/gpsimd queues, keeping Scalar for ACT.
 - outs emitted per (tile, i) right after t_i: steady out-DMA flow.
"""

import os

import numpy as np

B, N, ORDER, NCORES = 4096, 4096, 3, 8
ROWS = B // NCORES  # 512 rows per core
P = 128
NT = ROWS // P  # 4 partition-tiles per core
HALF = 2048

# per-(tile,dot) impl: v=STT+accum(DVE), s=TT+ACT(Scalar reduce)
DOTS = os.environ.get("CK_DOTS", "vss" "sss" "vss" "vss")
# per-(tile,out) impl: d=TS+TT(DVE), e=ACT-mul+TT, p=PE, g=ACT-mul+GpSimd-add
LANES = os.environ.get("CK_LANES", "ppp" "pdp" "dpd" "dpd")
# DMA issue queues: s=sync, a=scalar, g=gpsimd
X_Q = os.environ.get("CK_X_Q", "sgsg")          # per x tile
OUT_Q = os.environ.get("CK_OUT_Q", "sgsgsgsgsgsg")  # per out (tile*3+i)
CONST_Q = os.environ.get("CK_CONST_Q", "g")     # wb1/wb2/cbb loads
XBUFS = int(os.environ.get("CK_XBUFS", "4"))
OBUFS = int(os.environ.get("CK_OBUFS", "6"))
TBUFS = int(os.environ.get("CK_TBUFS", "3"))
PBUFS = int(os.environ.get("CK_PBUFS", "2"))

_prog_cache = {}


def _expand(s, n=None):
    s = "".join(c for c in s if not c.isspace())
    n = n or ORDER * NT
    if len(s) == 1:
        s = s * n
    if len(s) == ORDER and n == ORDER * NT:
        s = s * NT
    assert len(s) == n, s
    return s


def _build_program():
    from contextlib import ExitStack

    import concourse.bacc as bacc
    import concourse.mybir as mybir
    import concourse.tile as tile

    f32 = mybir.dt.float32
    f16 = mybir.dt.float16
    Alu = mybir.AluOpType
    Act = mybir.ActivationFunctionType

    dots = _expand(DOTS)
    lanes = _expand(LANES)
    x_q = _expand(X_Q, NT)
    out_q = _expand(OUT_Q)

    nc = bacc.Bacc("TRN2")
    xs = nc.dram_tensor("xs", [ROWS, N], f16, kind="ExternalInput")
    w3 = nc.dram_tensor("w3", [ORDER, N], f16, kind="ExternalInput")
    cb3 = nc.dram_tensor("cb3", [ORDER, N], f16, kind="ExternalInput")
    dd = nc.dram_tensor("dd", [P, ORDER], f32, kind="ExternalInput")
    eye = nc.dram_tensor("eye", [P, P], f16, kind="ExternalInput")
    out = nc.dram_tensor("out", [ORDER, ROWS, N], f16, kind="ExternalOutput")

    need_cbb = sorted(
        {
            q
            for k in range(NT)
            for q, c in enumerate(lanes[3 * k : 3 * k + 3])
            if c in "gdeh"
        }
    )
    any_pe_out = any(c == "p" for c in lanes)

    def q_eng(c):
        return {"s": nc.sync, "a": nc.scalar, "g": nc.gpsimd}[c]

    cq = q_eng(_expand(CONST_Q, 1))

    with ExitStack() as ctx:
        tc = ctx.enter_context(tile.TileContext(nc))
        consts = ctx.enter_context(tc.tile_pool(name="consts", bufs=1))
        xpool = ctx.enter_context(tc.tile_pool(name="xpool", bufs=XBUFS))
        small = ctx.enter_context(tc.tile_pool(name="small", bufs=4))
        opool = ctx.enter_context(tc.tile_pool(name="opool", bufs=OBUFS))
        tpool = ctx.enter_context(tc.tile_pool(name="tpool", bufs=TBUFS))
        psum = ctx.enter_context(tc.tile_pool(name="psum", bufs=PBUFS, space="PSUM"))
        scratchpool = ctx.enter_context(tc.tile_pool(name="scratch", bufs=1))

        wb = [
            consts.tile([P, N], f16, tag=f"wb{j}", name=f"wb{j}")
            for j in range(ORDER)
        ]
        cbb = {
            i: consts.tile([P, N], f16, tag=f"cbb{i}", name=f"cbb{i}")
            for i in need_cbb
        }
        dd_t = consts.tile([P, ORDER], f32, tag="dd")
        eye_t = consts.tile([P, P], f16, tag="eye")

        # Queue FIFO order == emission order. Lead-in priority: wb0 and x0
        # halves split across the sync/gpsimd queues so dot0 of tile 0 can
        # start as early as the ~11us queue warm-up allows; x tiles BEFORE
        # bulk consts (a tile stuck behind consts stalls DVE for ~20us).
        xts = [
            xpool.tile([P, N], f16, tag="x", name=f"x{k}") for k in range(NT)
        ]
        nc.sync.dma_start(
            out=wb[0][:, :HALF], in_=w3[0:1, :HALF].to_broadcast([P, HALF])
        )
        nc.gpsimd.dma_start(
            out=wb[0][:, HALF:], in_=w3[0:1, HALF:].to_broadcast([P, HALF])
        )
        nc.sync.dma_start(out=xts[0][:, :HALF], in_=xs[:P, :HALF])
        nc.gpsimd.dma_start(out=xts[0][:, HALF:], in_=xs[:P, HALF:])
        nc.scalar.dma_start(out=dd_t, in_=dd[:, :])
        cpack = None
        if any_pe_out:
            cpack = consts.tile([2 * 32 + 1, N], f16, tag="cpack")
            nc.scalar.dma_start(out=eye_t, in_=eye[:, :])
            for j in range(ORDER):
                nc.scalar.dma_start(
                    out=cpack[32 * j : 32 * j + 1, :], in_=cb3[j : j + 1, :]
                )
        nc.sync.dma_start(out=wb[1], in_=w3[1:2, :].to_broadcast([P, N]))
        nc.sync.dma_start(out=xts[1], in_=xs[P : 2 * P, :])
        nc.gpsimd.dma_start(out=wb[2], in_=w3[2:3, :].to_broadcast([P, N]))
        nc.gpsimd.dma_start(out=xts[2], in_=xs[2 * P : 3 * P, :])
        for n_, i in enumerate(need_cbb):
            q_eng("sg"[n_ % 2]).dma_start(
                out=cbb[i], in_=cb3[i : i + 1, :].to_broadcast([P, N])
            )
        nc.gpsimd.dma_start(out=xts[3], in_=xs[3 * P : 4 * P, :])
        opack = consts.tile([2 * 32 + 1, P], f16, tag="opack")
        nc.vector.memset(opack, 1.0)

        def row_of(pack, j):
            return pack[32 * j : 32 * j + 1, :]

        def emit_dot(k, j, x_t, cs, ts):
            scratch = scratchpool.tile([P, N], f16, tag="scr")
            cj = small.tile([P, 1], f32, tag=f"c{j}")
            if k == 0 and j == 0 and dots[0] == "v":
                cp = [
                    small.tile([P, 1], f32, tag=f"cp{h}", name=f"cp{h}")
                    for h in range(2)
                ]
                for h in range(2):
                    sl = slice(h * HALF, (h + 1) * HALF)
                    nc.vector.scalar_tensor_tensor(
                        out=scratch[:, sl],
                        in0=x_t[:, sl],
                        scalar=1.0,
                        in1=wb[0][:, sl],
                        op0=Alu.mult,
                        op1=Alu.mult,
                        accum_out=cp[h],
                    )
                nc.vector.tensor_add(cj, cp[0], cp[1])
            elif dots[3 * k + j] == "s":
                prod = tpool.tile([P, N], f16, tag="prod")
                nc.vector.tensor_tensor(prod, x_t, wb[j], Alu.mult)
                nc.scalar.activation(prod, prod, Act.Copy, accum_out=cj)
            else:
                nc.vector.scalar_tensor_tensor(
                    out=scratch,
                    in0=x_t,
                    scalar=1.0,
                    in1=wb[j],
                    op0=Alu.mult,
                    op1=Alu.mult,
                    accum_out=cj,
                )
            cs.append(cj)
            if j == 0:
                t1 = small.tile([P, 1], f32, tag="t0")
                nc.vector.tensor_scalar_add(t1, cs[0], 1.0)
                ts.append(t1)
            else:
                si = small.tile([P, 1], f32, tag=f"s{j}")
                nc.vector.tensor_scalar(
                    out=si,
                    in0=cs[j],
                    scalar1=ts[j - 1],
                    scalar2=dd_t[:, j : j + 1],
                    op0=Alu.mult,
                    op1=Alu.add,
                )
                ti = small.tile([P, 1], f32, tag=f"t{j}")
                nc.vector.tensor_add(ti, ts[j - 1], si)
                ts.append(ti)

        def emit_out(k, i, x_t, ti):
            rows = slice(k * P, (k + 1) * P)
            lane = lanes[3 * k + i]
            ob = opool.tile([P, N], f16, tag="ob")
            if lane == "h":
                for hh in range(2):
                    sl = slice(hh * HALF, (hh + 1) * HALF)
                    nc.vector.tensor_scalar(
                        out=ob[:, sl], in0=x_t[:, sl], scalar1=ti,
                        scalar2=None, op0=Alu.mult
                    )
                    nc.vector.tensor_add(ob[:, sl], ob[:, sl], cbb[i][:, sl])
                    q_eng("sg"[hh]).dma_start(
                        out=out[i, rows, sl], in_=ob[:, sl]
                    )
                return
            if lane == "d":
                nc.vector.tensor_scalar(
                    out=ob, in0=x_t, scalar1=ti, scalar2=None, op0=Alu.mult
                )
                nc.vector.tensor_add(ob, ob, cbb[i])
            elif lane == "e":
                nc.scalar.mul(ob, x_t, ti)
                nc.vector.tensor_add(ob, ob, cbb[i])
            elif lane == "g":
                tmp = tpool.tile([P, N], f16, tag="prod")
                nc.scalar.mul(tmp, x_t, ti)
                nc.gpsimd.tensor_add(ob, tmp, cbb[i])
            else:  # 'p'
                dg = small.tile([P, P], f16, tag=f"dg{i}")
                nc.vector.tensor_scalar_mul(dg, eye_t, ti)
                for h in range(N // HALF):
                    pt = psum.tile([P, HALF], f32, tag="ps")
                    for q in range(HALF // 512):
                        sl = slice(h * HALF + q * 512, h * HALF + (q + 1) * 512)
                        nc.tensor.matmul(
                            pt[:, q * 512 : (q + 1) * 512],
                            lhsT=row_of(opack, i),
                            rhs=row_of(cpack, i)[:, sl],
                            start=True,
                            stop=False,
                        )
                    for q in range(HALF // 512):
                        sl = slice(h * HALF + q * 512, h * HALF + (q + 1) * 512)
                        nc.tensor.matmul(
                            pt[:, q * 512 : (q + 1) * 512],
                            lhsT=dg,
                            rhs=x_t[:, sl],
                            start=False,
                            stop=True,
                        )
                    nc.scalar.copy(ob[:, h * HALF : (h + 1) * HALF], pt)
            q_eng(out_q[3 * k + i]).dma_start(out=out[i, rows, :], in_=ob)

        # Per-tile emission with a one-dot lead: an out's DVE/Scalar ops
        # never sit in a queue directly ahead of the next dot's reduce.
        for k in range(NT):
            cs, ts = [], []
            emit_dot(k, 0, xts[k], cs, ts)
            emit_dot(k, 1, xts[k], cs, ts)
            emit_out(k, 0, xts[k], ts[0])
            emit_dot(k, 2, xts[k], cs, ts)
            emit_out(k, 1, xts[k], ts[1])
            emit_out(k, 2, xts[k], ts[2])

    nc.finalize()
    return nc


def _get_program():
    if "nc" not in _prog_cache:
        _prog_cache["nc"] = _build_program()
    return _prog_cache["nc"]


def _prep_inputs(x, w, b):
    x16 = np.asarray(x, dtype=np.float32).astype(np.float16)
    w_r = np.asarray(w, dtype=np.float32).reshape(ORDER, N).astype(np.float16)
    b_r = np.asarray(b, dtype=np.float32).reshape(ORDER, N)
    cumb = np.cumsum(b_r, axis=0).astype(np.float16)  # cumb[i] = b_0 + ... + b_i
    d = np.zeros(ORDER, dtype=np.float64)
    for i in range(1, ORDER):
        d[i] = cumb[i - 1].astype(np.float64) @ w_r[i].astype(np.float64)
    dd = np.tile(d.astype(np.float32)[None, :], (P, 1))
    eye = np.eye(P, dtype=np.float16)

    shared = {
        "w3": np.ascontiguousarray(w_r),
        "cb3": np.ascontiguousarray(cumb),
        "dd": np.ascontiguousarray(dd),
        "eye": eye,
    }
    in_maps = []
    for c in range(NCORES):
        m = dict(shared)
        m["xs"] = np.ascontiguousarray(x16[c * ROWS : (c + 1) * ROWS, :])
        in_maps.append(m)
    return in_maps


def _run(x, w, b, trace=False):
    from concourse.bass_utils import run_bass_kernel_spmd

    nc = _get_program()
    in_maps = _prep_inputs(x, w, b)
    res = run_bass_kernel_spmd(nc, in_maps, core_ids=list(range(NCORES)), trace=trace)
    outs = [np.asarray(r["out"]) for r in res.results]  # each [ORDER, ROWS, N] f16
    full = np.concatenate(outs, axis=1)  # [ORDER, B, N]
    return (
        tuple(np.ascontiguousarray(full[i]).astype(np.float32) for i in range(ORDER)),
        res,
    )


def kernel(x, w, b):
    outs, _ = _run(x, w, b, trace=False)
    return outs
